# revision 1
# baseline (speedup 1.0000x reference)
"""CTSMamba Trainium2 kernel: GSC conv block + Mamba selective scan.

Self-contained: takes FULL inputs, shards across 8 NeuronCores internally
(spatial token sharding; 512 tokens = 2 D-slices per core), returns FULL output.
"""
import sys, os
for p in ("/opt/trn_rl_repo", "/root/.axon_site/_ro/trn_rl_repo"):
    if os.path.isdir(p) and p not in sys.path:
        sys.path.insert(0, p)

import numpy as np
import ml_dtypes
from contextlib import ExitStack

BFNP = ml_dtypes.bfloat16

# ---- problem constants ----
B, C, Dd, H, W = 4, 384, 16, 16, 16
N16, DCONV = 16, 4
DIN, R24 = 768, 24
EPS = 1e-5
L = 4096
NCORES = 8
SLAB, SPS = 512, 256
CT, DT = 3, 6
PADHW = 18
PSL = PADHW * PADHW
WSL = 4
WIN7 = WSL * PSL
E1, E2 = 768, 1280
NSEG, NG = 8, 2
SEGW = SLAB
GW = NSEG * SEGW
HLF = 384                      # half of E1 for psum-bank-sized matmuls


# ======================================================================
# host-side input prep
# ======================================================================

def _f32(a):
    return np.ascontiguousarray(np.asarray(a, np.float32))


def _bf(a):
    return np.ascontiguousarray(np.asarray(a, np.float32).astype(BFNP))


def _host_x_window(x, j):
    out = np.zeros((B, C, WSL, PADHW, PADHW), np.float32)
    for k in range(WSL):
        d = 2 * j - 1 + k
        if 0 <= d < Dd:
            out[:, :, k, 1:-1, 1:-1] = x[:, :, d]
    return out


def _host_masks(j):
    mE2 = np.zeros((E2,), np.float32)
    for k in range(5):
        if 0 <= 2 * j - 2 + k < Dd:
            mE2[k * SPS:(k + 1) * SPS] = 1.0
    mE1 = np.zeros((E1,), np.float32)
    for k in range(3):
        if 0 <= 2 * j - 1 + k < Dd:
            mE1[k * SPS:(k + 1) * SPS] = 1.0
    return mE2, mE1


def _conv_lhsT(w):
    out = np.zeros((CT, CT, 128, 27 * 128), np.float32)
    for o in range(27):
        dz, dy, dx = o // 9, (o // 3) % 3, o % 3
        wm = w[:, :, dz, dy, dx]
        for kt in range(CT):
            for mt in range(CT):
                out[kt, mt, :, o * 128:(o + 1) * 128] = \
                    wm[mt * 128:(mt + 1) * 128, kt * 128:(kt + 1) * 128].T
    return _bf(out)


def _mat_lhsT(wT, ktiles, mtiles, kp=128):
    K, M = wT.shape
    out = np.zeros((ktiles, mtiles, kp, 128), np.float32)
    for kt in range(ktiles):
        for mt in range(mtiles):
            blk = wT[kt * kp:(kt + 1) * kp, mt * 128:(mt + 1) * 128]
            out[kt, mt, :blk.shape[0], :blk.shape[1]] = blk
    return _bf(out)


def _xpw_lhsT(xpw):
    """x_proj_w: [56, DIN] -> lhsT [DT, 128, 64]: dt rows at 0-23, B at 32-47, C at 48-63."""
    out = np.zeros((DT, 128, 64), np.float32)
    wT = xpw.T                                    # [DIN, 56]
    for kt in range(DT):
        blk = wT[kt * 128:(kt + 1) * 128]
        out[kt, :, 0:R24] = blk[:, 0:R24]
        out[kt, :, 32:48] = blk[:, R24:R24 + N16]
        out[kt, :, 48:64] = blk[:, R24 + N16:]
    return _bf(out)


def _split_rows(v, ntiles):
    return _f32(v).reshape(ntiles, 128, 1)


def prep_inputs(inputs):
    x = _f32(inputs["x"])
    shared = {
        "w1T": _conv_lhsT(_f32(inputs["gsc_w1"])),
        "w2T": _conv_lhsT(_f32(inputs["gsc_w2"])),
        "w3T": _mat_lhsT(_f32(inputs["gsc_w3"])[:, :, 0, 0, 0].T, CT, CT),
        "w4T": _mat_lhsT(_f32(inputs["gsc_w4"])[:, :, 0, 0, 0].T, CT, CT),
        "b1": _split_rows(inputs["gsc_b1"], CT),
        "b2": _split_rows(inputs["gsc_b2"], CT),
        "b3": _split_rows(inputs["gsc_b3"], CT),
        "b4": _split_rows(inputs["gsc_b4"], CT),
        "lng": _split_rows(inputs["ln_g"], CT),
        "lnb": _split_rows(inputs["ln_b"], CT),
        "inwT": _mat_lhsT(_f32(inputs["in_proj_w"]).T, CT, 2 * DT),
        "c1w": _f32(inputs["conv1d_w"])[:, 0, :].reshape(DT, 128, DCONV),
        "c1b": _split_rows(inputs["conv1d_b"], DT),
        "xpwT": _xpw_lhsT(_f32(inputs["x_proj_w"])),
        "dtwT": np.ascontiguousarray(
            _mat_lhsT(_f32(inputs["dt_proj_w"]).T, 1, DT, kp=R24)[0]),
        "dtb": _split_rows(inputs["dt_proj_b"], DT),
        "aneg": _f32(-np.exp(_f32(inputs["A_log"]))).reshape(DT, 128, N16),
        "dsk": _split_rows(inputs["D_skip"], DT),
        "owT": _mat_lhsT(_f32(inputs["out_proj_w"]).T, DT, CT),
    }
    per_core = []
    for j in range(NCORES):
        xw = _host_x_window(x, j)
        mE2, mE1 = _host_masks(j)
        m = dict(shared)
        m["xw"] = _bf(xw.reshape(B, CT, 128, WIN7))
        m["mE2"] = _bf(np.broadcast_to(mE2, (128, E2)))
        m["mE1"] = _bf(np.broadcast_to(mE1, (128, E1)))
        per_core.append(m)
    return per_core


# ======================================================================
# device kernel build
# ======================================================================

_BUILT = {}


def build_nc(debug=False, nbatch=B, apow=False):
    key = ("k", debug, nbatch, apow)
    if key in _BUILT:
        return _BUILT[key]
    from concourse import bass, bacc, tile, mybir

    F32, BF16 = mybir.dt.float32, mybir.dt.bfloat16
    AF = mybir.ActivationFunctionType
    OP = mybir.AluOpType
    ET = mybir.EngineType

    nc = bacc.Bacc("TRN2", num_devices=NCORES, debug=False)

    di = {}
    def din(name, shape, dt=BF16):
        di[name] = nc.dram_tensor(name, list(shape), dt, kind="ExternalInput")

    din("xw", (B, CT, 128, WIN7))
    din("mE2", (128, E2)); din("mE1", (128, E1))
    din("w1T", (CT, CT, 128, 27 * 128)); din("w2T", (CT, CT, 128, 27 * 128))
    din("w3T", (CT, CT, 128, 128)); din("w4T", (CT, CT, 128, 128))
    for nm in ("b1", "b2", "b3", "b4", "lng", "lnb"):
        din(nm, (CT, 128, 1), F32)
    din("inwT", (CT, 2 * DT, 128, 128))
    din("c1w", (DT, 128, DCONV), F32); din("c1b", (DT, 128, 1), F32)
    din("xpwT", (DT, 128, 64))
    din("dtwT", (DT, R24, 128))
    din("dtb", (DT, 128, 1), F32)
    din("aneg", (DT, 128, N16), F32)
    din("dsk", (DT, 128, 1), F32)
    din("owT", (DT, CT, 128, 128))
    out_t = nc.dram_tensor("out", [B, CT, 128, SLAB], F32, kind="ExternalOutput")
    dbg = {}
    if debug:
        for nm, shape, dt in (("d_x1n", (CT, 128, E2), BF16),
                              ("d_xg", (CT, 128, E1), BF16),
                              ("d_xn", (CT, 128, E1), BF16),
                              ("d_u", (DT, 128, SLAB), BF16),
                              ("d_dt", (DT, 128, SLAB), F32),
                              ("d_dbl", (56, SLAB), F32),
                              ("d_y", (DT, 128, SLAB), BF16),
                              ("d_h0", (DT, 128, N16), F32),
                              ("d_lnrow", (1, 2 * E1), BF16),
                              ("d_mub", (2, 128, E1), BF16)):
            dbg[nm] = nc.dram_tensor(nm, list(shape), dt, kind="ExternalOutput")

    with tile.TileContext(nc, num_cores=NCORES) as tc:
      with ExitStack() as ctx:
        P = lambda name, bufs=1, **kw: ctx.enter_context(
            tc.tile_pool(name=name, bufs=bufs, **kw))
        wpool = P("wts", 1)
        wstr = P("wstr", 2)
        xwp = P("xwp", 1)
        act = P("act", 1)
        sm = P("small", 1)
        scn = P("scan", 1)
        psP = P("psP", 1, space="PSUM")
        drp = P("dram", 1, space="DRAM")

        def load1(name, shape, dt=BF16, src=None):
            t = wpool.tile(list(shape), dt, tag=name, name=name)
            nc.sync.dma_start(t[:], (src if src is not None else di[name])[:])
            return t

        w3 = [[load1(f"w3_{k}_{m}", (128, 128), BF16, di["w3T"][k, m])
               for m in range(CT)] for k in range(CT)]
        w4 = [[load1(f"w4_{k}_{m}", (128, 128), BF16, di["w4T"][k, m])
               for m in range(CT)] for k in range(CT)]
        ow = [[load1(f"ow_{k}_{m}", (128, 128), BF16, di["owT"][k, m])
               for m in range(CT)] for k in range(DT)]
        xpw = [load1(f"xpw_{k}", (128, 64), BF16, di["xpwT"][k]) for k in range(DT)]
        dtw = [load1(f"dtw_{m}", (R24, 128), BF16, di["dtwT"][m]) for m in range(DT)]
        b_ = {nm: [load1(f"{nm}_{t}", (128, 1), F32, di[nm][t]) for t in range(CT)]
              for nm in ("b1", "b2", "b3", "b4", "lng", "lnb")}
        c1w = [load1(f"c1w_{t}", (128, DCONV), F32, di["c1w"][t]) for t in range(DT)]
        c1b = [load1(f"c1b_{t}", (128, 1), F32, di["c1b"][t]) for t in range(DT)]
        dtb = [load1(f"dtb_{t}", (128, 1), F32, di["dtb"][t]) for t in range(DT)]
        aneg = [load1(f"aneg_{t}", (128, N16), F32, di["aneg"][t]) for t in range(DT)]
        dsk = [load1(f"dsk_{t}", (128, 1), F32, di["dsk"][t]) for t in range(DT)]
        mE2 = load1("mE2", (128, E2), BF16)
        mE1 = load1("mE1", (128, E1), BF16)
        ones1 = wpool.tile([128, 1], BF16, tag="ones1")
        nc.vector.memset(ones1[:], 1.0)
        epsc = wpool.tile([128, 1], F32, tag="epsc")
        nc.vector.memset(epsc[:], EPS)
        zseg = wpool.tile([128, SLAB], BF16, tag="zseg")
        nc.vector.memset(zseg[:], 0.0)

        pid = nc.partition_id()
        jm_reg = nc.alloc_register(ET.SP)
        nc.sync.reg_alu(jm_reg, pid, 1, OP.subtract)
        nc.sync.reg_alu(jm_reg, jm_reg, 0, OP.max)
        jp_reg = nc.alloc_register(ET.SP)
        nc.sync.reg_alu(jp_reg, pid, 1, OP.add)
        nc.sync.reg_alu(jp_reg, jp_reg, NCORES - 1, OP.min)
        jm = nc.snap(jm_reg, min_val=0, max_val=NCORES - 1)
        jp = nc.snap(jp_reg, min_val=0, max_val=NCORES - 1)

        RG = [list(range(NCORES))]

        def norm_consts6(red, rinv6, shift6, scr):
            """red [128, 2*CT] (sum,sq interleaved) -> rinv6/shift6 [128, CT]."""
            rv = red[:].rearrange("p (c two) -> p c two", two=2)
            m = scr[:, 0:CT]; q = scr[:, CT:2 * CT]
            nc.vector.tensor_scalar(m, rv[:, :, 0:1].rearrange("p c t -> p (c t)"),
                                    1.0 / L, 0.0, OP.mult, OP.add)
            nc.vector.tensor_scalar(q, rv[:, :, 1:2].rearrange("p c t -> p (c t)"),
                                    1.0 / L, 0.0, OP.mult, OP.add)
            nc.vector.tensor_tensor(scr[:, 2 * CT:3 * CT], m, m, OP.mult)
            nc.vector.tensor_tensor(q, q, scr[:, 2 * CT:3 * CT], OP.subtract)
            nc.scalar.activation(q, q, AF.Ln, bias=epsc[:])
            nc.scalar.activation(rinv6[:], q, AF.Exp, scale=-0.5)
            nc.vector.tensor_tensor(shift6[:], m, rinv6[:], OP.mult)

        def _powers(dA, base, g, pw):
            """Fill 8-segment group tile dA with base^(g*8+s+1), s=0..7."""
            p2, p4, p8 = pw[:, 0:SLAB], pw[:, SLAB:2 * SLAB], pw[:, 2 * SLAB:]
            sg = lambda s: dA[:, s * SEGW:(s + 1) * SEGW]
            TT = lambda o, a, c: nc.vector.tensor_tensor(o, a, c, OP.mult)
            TT(p2, base[:], base[:])
            TT(p4, p2, p2)
            TT(p8, p4, p4)
            if g == 0:
                nc.vector.tensor_copy(sg(0), base[:])
                nc.vector.tensor_copy(sg(1), p2)
                TT(sg(2), p2, base[:])
                nc.vector.tensor_copy(sg(3), p4)
                TT(sg(4), p4, base[:])
                TT(sg(5), p4, p2)
                TT(sg(6), sg(5), base[:])
                nc.vector.tensor_copy(sg(7), p8)
            else:
                TT(sg(0), p8, base[:])
                TT(sg(1), p8, p2)
                TT(sg(2), sg(1), base[:])
                TT(sg(3), p8, p4)
                TT(sg(4), sg(3), base[:])
                TT(sg(5), sg(3), p2)
                TT(sg(6), sg(5), base[:])
                TT(sg(7), p8, p8)

        def _silu(out_ap, in_ap):
            """silu(x) = x / (1 + exp(-x)) using only the exp table + DVE."""
            e = sm.tile([128, SLAB], F32, tag="silue")
            nc.scalar.activation(e[:], in_ap, AF.Exp, scale=-1.0)
            nc.vector.tensor_scalar(e[:], e[:], 1.0, None, OP.add)
            nc.vector.reciprocal(e[:], e[:])
            nc.vector.tensor_tensor(out_ap, in_ap, e[:], OP.mult)

        def stats_roundtrip(tag, b, pk, W_):
            """pk [128, W_] already packed -> AG -> tree-summed [128, W_]."""
            bi = drp.tile([128, W_], F32, tag=f"stb_{tag}_{b}",
                          name=f"stb_{tag}_{b}")
            bo = drp.tile([NCORES, 128, W_], F32, tag=f"stbo_{tag}_{b}",
                          name=f"stbo_{tag}_{b}", addr_space="Shared")
            nc.sync.dma_start(bi[:], pk[:])
            nc.gpsimd.collective_compute("AllGather", OP.bypass, replica_groups=RG,
                                         ins=[bi.opt()], outs=[bo.opt()])
            allst = sm.tile([128, NCORES * W_], F32, tag=f"stall",
                            name=f"stall_{tag}_{b}")
            nc.sync.dma_start(
                allst[:].rearrange("p (r c) -> p r c", r=NCORES),
                bo[:].rearrange("r p c -> p r c"))
            for half in (4, 2, 1):
                nc.vector.tensor_tensor(allst[:, 0:half * W_],
                                        allst[:, 0:half * W_],
                                        allst[:, half * W_:2 * half * W_], OP.add)
            red = sm.tile([128, W_], F32, tag=f"stred_{tag}",
                          name=f"stred_{tag}_{b}")
            nc.vector.tensor_copy(red[:], allst[:, 0:W_])
            return red

        for b in range(nbatch):
            # ---------------- x window + flat E1 ----------------
            xw = [xwp.tile([128, WIN7], BF16, tag=f"xw{ct}", name=f"xw{ct}_{b}")
                  for ct in range(CT)]
            for ct in range(CT):
                nc.sync.dma_start(xw[ct][:], di["xw"][b, ct])
            xwv = [t[:].rearrange("p (d h w) -> p d h w", d=WSL, h=PADHW, w=PADHW)
                   for t in xw]
            xfE1 = []
            for ct in range(CT):
                t = act.tile([128, E1], BF16, tag=f"xf{ct}", name=f"xf{ct}_{b}")
                nc.vector.tensor_copy(
                    t[:].rearrange("p (s h w) -> p s h w", s=3, h=H, w=W),
                    xwv[ct][:, 0:3, 1:17, 1:17])
                xfE1.append(t)

            # ---------------- conv1 on slab ----------------
            c1raw = []
            pk1 = sm.tile([128, CT * 2], F32, tag="stpk1")
            for mt in range(CT):
                psum = psP.tile([128, SLAB], F32, tag="convps", bufs=2)
                first = True
                for kt in range(CT):
                    wsl = wstr.tile([128, 27 * 128], BF16, tag="wslot")
                    nc.sync.dma_start(wsl[:], di["w1T"][kt, mt])
                    for o in range(27):
                        dz, dy, dx = o // 9, (o // 3) % 3, o % 3
                        rhs = xwv[kt][:, dz:2 + dz, dy:dy + H, dx:dx + W]
                        nc.tensor.matmul(psum[:], wsl[:, o * 128:(o + 1) * 128],
                                         rhs, start=first,
                                         stop=(kt == CT - 1 and o == 26))
                        first = False
                raw = act.tile([128, SLAB], BF16, tag=f"c1r{mt}")
                scr = act.tile([128, SLAB], BF16, tag="sqscr")
                nc.vector.tensor_scalar(raw[:], psum[:], b_["b1"][mt][:], 0.0,
                                        OP.add, OP.add,
                                        accum_out=pk1[:, 2 * mt:2 * mt + 1])
                nc.scalar.activation(scr[:], psum[:], AF.Square,
                                     bias=b_["b1"][mt][:],
                                     accum_out=pk1[:, 2 * mt + 1:2 * mt + 2])
                c1raw.append(raw)

            ag1i = drp.tile([CT, 128, SLAB], BF16, tag=f"ag1i_{b}")
            ag1o = drp.tile([NCORES, CT, 128, SLAB], BF16, tag=f"ag1o_{b}",
                            addr_space="Shared")
            for ct in range(CT):
                nc.sync.dma_start(ag1i[ct], c1raw[ct][:])
            nc.gpsimd.collective_compute("AllGather", OP.bypass, replica_groups=RG,
                                         ins=[ag1i.opt()], outs=[ag1o.opt()])
            st1 = stats_roundtrip("c1", b, pk1, CT * 2)
            ri1 = sm.tile([128, CT], F32, tag="ri1")
            sh1 = sm.tile([128, CT], F32, tag="sh1")
            scrN = sm.tile([128, 4 * CT], F32, tag="nscr")
            norm_consts6(st1, ri1, sh1, scrN)

            # ---------------- x1n: gather E2, norm+relu+mask into padded ----------------
            x1pad, x1pv = [], []
            for ct in range(CT):
                e2 = act.tile([128, E2], BF16, tag=f"e2_{ct}")
                nc.sync.dma_start(e2[:, 0:SLAB],
                                  ag1o[bass.ds(jm, 1), ct].squeeze(0))
                nc.vector.tensor_copy(e2[:, SLAB:2 * SLAB], c1raw[ct][:])
                nc.sync.dma_start(
                    e2[:, 2 * SLAB:2 * SLAB + SPS],
                    ag1o[bass.ds(jp, 1), ct, :, 0:SPS].squeeze(0))
                pad = act.tile([128, 5 * PSL], BF16, tag=f"x1p{ct}")
                nc.gpsimd.memset(pad[:], 0.0)
                pv = pad[:].rearrange("p (s h w) -> p s h w", s=5, h=PADHW, w=PADHW)
                pint = pv[:, :, 1:17, 1:17]
                e2v = e2[:].rearrange("p (s h w) -> p s h w", s=5, h=H, w=W)
                m2v = mE2[:].rearrange("p (s h w) -> p s h w", s=5, h=H, w=W)
                nc.vector.tensor_scalar(pint, e2v, ri1[:, ct:ct + 1],
                                        sh1[:, ct:ct + 1], OP.mult, OP.subtract)
                nc.vector.tensor_scalar_max(pint, pint, 0.0)
                nc.vector.tensor_tensor(pint, pint, m2v, OP.mult)
                if debug and b == 0:
                    xbf = act.tile([128, E2], BF16, tag=f"e2_{ct}", name=f"dbgx1n{ct}")
                    nc.vector.tensor_copy(
                        xbf[:].rearrange("p (s h w) -> p s h w", s=5, h=H, w=W),
                        pint)
                    nc.sync.dma_start(dbg["d_x1n"][ct], xbf[:])
                x1pad.append(pad); x1pv.append(pv)

            # ---------------- conv2 (slab) + conv3 (E1) ----------------
            c2raw, c3raw = [], []
            pk23 = sm.tile([128, CT * 4], F32, tag="stpk23")
            for mt in range(CT):
                psum = psP.tile([128, SLAB], F32, tag="convps", bufs=2)
                first = True
                for kt in range(CT):
                    wsl = wstr.tile([128, 27 * 128], BF16, tag="wslot")
                    nc.sync.dma_start(wsl[:], di["w2T"][kt, mt])
                    for o in range(27):
                        dz, dy, dx = o // 9, (o // 3) % 3, o % 3
                        rhs = x1pv[kt][:, 1 + dz:3 + dz, dy:dy + H, dx:dx + W]
                        nc.tensor.matmul(psum[:], wsl[:, o * 128:(o + 1) * 128],
                                         rhs, start=first,
                                         stop=(kt == CT - 1 and o == 26))
                        first = False
                raw = act.tile([128, SLAB], BF16, tag=f"c2r{mt}")
                scr = act.tile([128, SLAB], BF16, tag="sqscr")
                nc.vector.tensor_scalar(raw[:], psum[:], b_["b2"][mt][:], 0.0,
                                        OP.add, OP.add,
                                        accum_out=pk23[:, 2 * mt:2 * mt + 1])
                nc.scalar.activation(scr[:], psum[:], AF.Square,
                                     bias=b_["b2"][mt][:],
                                     accum_out=pk23[:, 2 * mt + 1:2 * mt + 2])
                c2raw.append(raw)

                raw3 = act.tile([128, E1], BF16, tag=f"c3r{mt}")
                s3a = sm.tile([128, 4], F32, tag="c3sa", name=f"c3sa{mt}_{b}")
                for half in range(2):
                    cols = slice(half * HLF, (half + 1) * HLF)
                    ph = psP.tile([128, HLF], F32, tag="hps", bufs=2)
                    for kt in range(CT):
                        nc.tensor.matmul(ph[:], w3[kt][mt], xfE1[kt][:, cols],
                                         start=(kt == 0), stop=(kt == CT - 1))
                    nc.vector.tensor_scalar(raw3[:, cols], ph[:], b_["b3"][mt][:],
                                            0.0, OP.add, OP.add)
                    scr3 = act.tile([128, HLF], BF16, tag="sqscr", name=f"sq3{mt}_{half}_{b}")
                    sl = slice(SPS, HLF) if half == 0 else slice(0, HLF)
                    nc.vector.tensor_scalar(scr3[:, sl], ph[:, sl],
                                            b_["b3"][mt][:], 0.0, OP.add, OP.add,
                                            accum_out=s3a[:, 2 * half:2 * half + 1])
                    nc.scalar.activation(scr3[:, sl], ph[:, sl], AF.Square,
                                         bias=b_["b3"][mt][:],
                                         accum_out=s3a[:, 2 * half + 1:2 * half + 2])
                nc.vector.tensor_tensor(
                    pk23[:, 2 * CT + 2 * mt:2 * CT + 2 * mt + 2],
                    s3a[:, 0:2], s3a[:, 2:4], OP.add)
                c3raw.append(raw3)

            ag2i = drp.tile([CT, 128, SLAB], BF16, tag=f"ag2i_{b}")
            ag2o = drp.tile([NCORES, CT, 128, SLAB], BF16, tag=f"ag2o_{b}",
                            addr_space="Shared")
            for ct in range(CT):
                nc.sync.dma_start(ag2i[ct], c2raw[ct][:])
            nc.gpsimd.collective_compute("AllGather", OP.bypass, replica_groups=RG,
                                         ins=[ag2i.opt()], outs=[ag2o.opt()])
            st23 = stats_roundtrip("c23", b, pk23, CT * 4)
            ri2 = sm.tile([128, CT], F32, tag="ri2")
            sh2 = sm.tile([128, CT], F32, tag="sh2")
            ri3 = sm.tile([128, CT], F32, tag="ri3")
            sh3 = sm.tile([128, CT], F32, tag="sh3")
            scrN = sm.tile([128, 4 * CT], F32, tag="nscr")
            norm_consts6(st23[:, 0:2 * CT], ri2, sh2, scrN)
            norm_consts6(st23[:, 2 * CT:], ri3, sh3, scrN)

            # ---------------- x1b + x2 -> xs on E1 ----------------
            xs_ = []
            for ct in range(CT):
                e1 = act.tile([128, E1], BF16, tag=f"e2_{ct}", name=f"c2e1_{ct}_{b}")
                nc.sync.dma_start(
                    e1[:, 0:SPS],
                    ag2o[bass.ds(jm, 1), ct, :, SPS:].squeeze(0))
                nc.vector.tensor_copy(e1[:, SPS:], c2raw[ct][:])
                nc.vector.tensor_scalar(e1[:], e1[:], ri2[:, ct:ct + 1],
                                        sh2[:, ct:ct + 1], OP.mult, OP.subtract)
                nc.vector.tensor_scalar_max(e1[:], e1[:], 0.0)
                x2 = act.tile([128, E1], BF16, tag=f"x2_{ct}")
                nc.vector.tensor_scalar(x2[:], c3raw[ct][:], ri3[:, ct:ct + 1],
                                        sh3[:, ct:ct + 1], OP.mult, OP.subtract)
                nc.vector.tensor_scalar_max(x2[:], x2[:], 0.0)
                nc.vector.tensor_tensor(x2[:], e1[:], x2[:], OP.add)
                nc.vector.tensor_tensor(x2[:], x2[:], mE1[:], OP.mult)
                xs_.append(x2)

            # ---------------- conv4 on E1 ----------------
            c4raw = []
            pk4 = sm.tile([128, CT * 2], F32, tag="stpk4")
            for mt in range(CT):
                raw4 = act.tile([128, E1], BF16, tag=f"c4r{mt}", bufs=2)
                s4a = sm.tile([128, 4], F32, tag="c3sa", name=f"c4sa{mt}_{b}")
                for half in range(2):
                    cols = slice(half * HLF, (half + 1) * HLF)
                    ph = psP.tile([128, HLF], F32, tag="hps", bufs=2)
                    for kt in range(CT):
                        nc.tensor.matmul(ph[:], w4[kt][mt], xs_[kt][:, cols],
                                         start=(kt == 0), stop=(kt == CT - 1))
                    nc.vector.tensor_scalar(raw4[:, cols], ph[:], b_["b4"][mt][:],
                                            0.0, OP.add, OP.add)
                    scr4 = act.tile([128, HLF], BF16, tag="sqscr", name=f"sq4{mt}_{half}_{b}")
                    sl = slice(SPS, HLF) if half == 0 else slice(0, HLF)
                    nc.vector.tensor_scalar(scr4[:, sl], ph[:, sl],
                                            b_["b4"][mt][:], 0.0, OP.add, OP.add,
                                            accum_out=s4a[:, 2 * half:2 * half + 1])
                    nc.scalar.activation(scr4[:, sl], ph[:, sl], AF.Square,
                                         bias=b_["b4"][mt][:],
                                         accum_out=s4a[:, 2 * half + 1:2 * half + 2])
                nc.vector.tensor_tensor(pk4[:, 2 * mt:2 * mt + 2],
                                        s4a[:, 0:2], s4a[:, 2:4], OP.add)
                c4raw.append(raw4)
            st4 = stats_roundtrip("c4", b, pk4, CT * 2)
            ri4 = sm.tile([128, CT], F32, tag="ri4")
            sh4 = sm.tile([128, CT], F32, tag="sh4")
            scrN = sm.tile([128, 4 * CT], F32, tag="nscr")
            norm_consts6(st4, ri4, sh4, scrN)

            # ---------------- xg (in c4raw tiles) + LN ----------------
            xg_ = []
            for mt in range(CT):
                xg = c4raw[mt]
                nc.vector.tensor_scalar(xg[:], xg[:], ri4[:, mt:mt + 1],
                                        sh4[:, mt:mt + 1], OP.mult, OP.subtract)
                nc.vector.tensor_scalar_max(xg[:], xg[:], 0.0)
                nc.vector.tensor_tensor(xg[:], xg[:], xfE1[mt][:], OP.add)
                xg_.append(xg)
                if debug and b == 0:
                    nc.sync.dma_start(dbg["d_xg"][mt], xg[:])
            lnph = [psP.tile([1, HLF], F32, tag=f"lnp{h}", bufs=1,
                             name=f"lnp{h}_{b}") for h in range(2)]
            for mt in range(CT):
                for half in range(2):
                    cols = slice(half * HLF, (half + 1) * HLF)
                    nc.tensor.matmul(lnph[half][:], ones1[:], xg_[mt][:, cols],
                                     start=(mt == 0), stop=(mt == CT - 1))
            mu_r = sm.tile([1, E1], F32, tag="mu_r")
            for half in range(2):
                cols = slice(half * HLF, (half + 1) * HLF)
                nc.vector.tensor_scalar(mu_r[:, cols], lnph[half][:], 1.0 / C, 0.0,
                                        OP.mult, OP.add)
            lnqh = [psP.tile([1, HLF], F32, tag=f"lnp{h}", bufs=1,
                             name=f"lnq{h}_{b}") for h in range(2)]
            for mt in range(CT):
                sq = act.tile([128, E1], BF16, tag="c3r0", name=f"lnsq{mt}_{b}")
                nc.scalar.activation(sq[:], xg_[mt][:], AF.Square)
                for half in range(2):
                    cols = slice(half * HLF, (half + 1) * HLF)
                    nc.tensor.matmul(lnqh[half][:], ones1[:], sq[:, cols],
                                     start=(mt == 0), stop=(mt == CT - 1))
            ri_r = sm.tile([1, E1], F32, tag="ri_r")
            scr_r = sm.tile([1, E1], F32, tag="scr_r")
            for half in range(2):
                cols = slice(half * HLF, (half + 1) * HLF)
                nc.vector.tensor_scalar(scr_r[:, cols], lnqh[half][:], 1.0 / C, 0.0,
                                        OP.mult, OP.add)
            nc.vector.tensor_tensor(ri_r[:], mu_r[:], mu_r[:], OP.mult)
            nc.vector.tensor_tensor(scr_r[:], scr_r[:], ri_r[:], OP.subtract)
            nc.scalar.activation(ri_r[:], scr_r[:], AF.Ln, bias=epsc[0:1, :])
            nc.scalar.activation(ri_r[:], ri_r[:], AF.Exp, scale=-0.5)
            mu_rb = sm.tile([1, 2 * E1], BF16, tag="mu_rb")
            nc.vector.tensor_copy(mu_rb[:, 0:E1], mu_r[:])
            nc.vector.tensor_copy(mu_rb[:, E1:], ri_r[:])
            lnd = drp.tile([1, 2 * E1], BF16, tag=f"lnd_{b}")
            nc.sync.dma_start(lnd[:], mu_rb[:])
            mu_b = sm.tile([128, E1], BF16, tag="mu_b")
            ri_b = sm.tile([128, E1], BF16, tag="ri_b")
            nc.sync.dma_start(mu_b[:], lnd[0:1, 0:E1].to_broadcast((128, E1)))
            nc.sync.dma_start(ri_b[:], lnd[0:1, E1:].to_broadcast((128, E1)))
            if debug and b == 0:
                nc.sync.dma_start(dbg["d_lnrow"][:], mu_rb[:])
                nc.sync.dma_start(dbg["d_mub"][0], mu_b[:])
                nc.sync.dma_start(dbg["d_mub"][1], ri_b[:])

            for ct in range(CT):
                xn = xg_[ct]
                nc.vector.tensor_tensor(xn[:], xn[:], mu_b[:], OP.subtract)
                nc.vector.tensor_tensor(xn[:], xn[:], ri_b[:], OP.mult)
                nc.vector.tensor_scalar(xn[:], xn[:], b_["lng"][ct][:],
                                        b_["lnb"][ct][:], OP.mult, OP.add)
                nc.vector.tensor_tensor(xn[:], xn[:], mE1[:], OP.mult)
                if debug and b == 0:
                    nc.sync.dma_start(dbg["d_xn"][ct], xn[:])
            xn_ = xg_

            # ---------------- in_proj (stream weights) ----------------
            u_, g_, z_ = [], [], []
            for mt in range(2 * DT):
                if mt < DT:
                    dst = act.tile([128, E1], BF16, tag="xin", bufs=2,
                                   name=f"xin{mt}_{b}")
                else:
                    dst = act.tile([128, SLAB], BF16, tag=f"z{mt - DT}")
                    z_.append(dst)
                for half in range(2):
                    cols = slice(half * HLF, (half + 1) * HLF)
                    ph = psP.tile([128, HLF], F32, tag="hps", bufs=2)
                    for kt in range(CT):
                        wi = wstr.tile([128, 128], BF16, tag="wislot")
                        nc.sync.dma_start(wi[:], di["inwT"][kt, mt])
                        nc.tensor.matmul(ph[:], wi[:], xn_[kt][:, cols],
                                         start=(kt == 0), stop=(kt == CT - 1))
                    if mt < DT:
                        nc.scalar.copy(dst[:, cols], ph[:])
                    else:
                        if half == 0:
                            nc.scalar.copy(dst[:, 0:HLF - SPS], ph[:, SPS:])
                        else:
                            nc.scalar.copy(dst[:, HLF - SPS:], ph[:])
                if mt < DT:
                    dt_ = mt
                    uacc = sm.tile([128, SLAB], F32, tag="uacc")
                    nc.vector.tensor_scalar(uacc[:], dst[:, SPS - 3:SPS - 3 + SLAB],
                                            c1w[dt_][:, 0:1], c1b[dt_][:],
                                            OP.mult, OP.add)
                    for k in range(1, DCONV):
                        nc.vector.scalar_tensor_tensor(
                            uacc[:], dst[:, SPS - 3 + k:SPS - 3 + k + SLAB],
                            c1w[dt_][:, k:k + 1], uacc[:], OP.mult, OP.add)
                    u = act.tile([128, SLAB], BF16, tag=f"u{dt_}")
                    _silu(u[:], uacc[:])
                    u_.append(u)
                    if debug and b == 0:
                        nc.sync.dma_start(dbg["d_u"][dt_], u[:])
                else:
                    _silu(dst[:], dst[:])
                    g_.append(dst)

            # ---------------- x_proj -> dbl ----------------
            dblp = psP.tile([64, SLAB], F32, tag="dblps", bufs=1)
            for kt in range(DT):
                nc.tensor.matmul(dblp[:], xpw[kt][:], u_[kt][:],
                                 start=(kt == 0), stop=(kt == DT - 1))
            dbl = sm.tile([64, SLAB], F32, tag="dbl")
            nc.scalar.copy(dbl[:], dblp[:])
            dblb = sm.tile([R24, SLAB], BF16, tag="dblb")
            nc.vector.tensor_copy(dblb[:], dbl[0:R24, :])
            if debug and b == 0:
                nc.sync.dma_start(dbg["d_dbl"][0:R24], dbl[0:R24, :])
                nc.sync.dma_start(dbg["d_dbl"][R24:R24 + N16], dbl[32:48, :])
                nc.sync.dma_start(dbg["d_dbl"][R24 + N16:], dbl[48:64, :])
            bc_bf = sm.tile([2 * N16, SLAB], BF16, tag="bcbf")
            nc.vector.tensor_copy(bc_bf[:], dbl[32:64, :])
            bcd = drp.tile([2 * N16, SLAB], BF16, tag=f"bcd_{b}")
            nc.sync.dma_start(bcd[:], bc_bf[:])

            # ---------------- dt chain ----------------
            dtu_t, cds_t, dtbf_t = [], [], []
            FDpk = sm.tile([128, DT * 32], F32, tag="FDpk")
            for dt_ in range(DT):
                psum = psP.tile([128, SLAB], F32, tag="dtps", bufs=1)
                nc.tensor.matmul(psum[:], dtw[dt_][:], dblb[:], start=True, stop=True)
                dtt = sm.tile([128, SLAB], F32, tag="dtexp", bufs=2,
                              name=f"dtt{dt_}_{b}")
                nc.scalar.activation(dtt[:], psum[:], AF.Exp, bias=dtb[dt_][:])
                cds = sm.tile([128, 1], F32, tag=f"cds{dt_}")
                nc.scalar.activation(dtt[:], dtt[:], AF.Ln, bias=1.0, accum_out=cds[:])
                cds_t.append(cds)
                if debug and b == 0:
                    nc.sync.dma_start(dbg["d_dt"][dt_], dtt[:])
                dtu = sm.tile([128, SLAB], BF16, tag=f"dtu{dt_}")
                nc.vector.tensor_tensor(dtu[:], dtt[:], u_[dt_][:], OP.mult)
                dtu_t.append(dtu)
                Dt0 = sm.tile([128, N16], F32, tag="Dt0")
                nc.vector.tensor_scalar(Dt0[:], aneg[dt_][:], cds[:], 0.0,
                                        OP.mult, OP.add)
                nc.scalar.activation(FDpk[:, dt_ * 32 + 16:dt_ * 32 + 32],
                                     Dt0[:], AF.Exp)
                dtbf = sm.tile([128, SLAB], BF16, tag=f"dtbf{dt_}")
                if apow:
                    # w = exp(-dt); dA_n = w^(n+1) via binary powers
                    nc.scalar.activation(dtbf[:], dtt[:], AF.Exp, scale=-1.0)
                else:
                    nc.vector.tensor_copy(dtbf[:], dtt[:])
                dtbf_t.append(dtbf)

            # ---------------- phase 1: local scans per group ----------------
            Wt_t = []
            y_ = [sm.tile([128, SLAB], BF16, tag=f"y{dt_}", name=f"y{dt_}_{b}")
                  for dt_ in range(DT)]
            seg = lambda t_, s: t_[:, s * SEGW:(s + 1) * SEGW]
            for g in range(NG):
                BBt = scn.tile([128, GW], BF16, tag="bc", name=f"BB{g}_{b}")
                CCt = scn.tile([128, GW], BF16, tag="cc", name=f"CC{g}_{b}", bufs=1)
                for s in range(NSEG):
                    n = g * NSEG + s
                    nc.sync.dma_start(seg(BBt, s),
                                      bcd[n:n + 1, :].to_broadcast((128, SEGW)))
                    nc.sync.dma_start(seg(CCt, s),
                                      bcd[N16 + n:N16 + n + 1, :].to_broadcast((128, SEGW)))
                for dt_ in range(DT):
                    if g == 0:
                        # W = inclusive cumprod of w (fp32 internal state)
                        Wt = sm.tile([128, SLAB], BF16, tag=f"Wt{dt_}")
                        nc.vector.tensor_tensor_scan(Wt[:], dtbf_t[dt_][:], zseg[:],
                                                     1.0, OP.mult, OP.add)
                        Wt_t.append(Wt)
                    dA = scn.tile([128, GW], BF16, tag="dA", bufs=1,
                                  name=f"dA{dt_}_{g}_{b}")
                    if apow:
                        pw = sm.tile([128, 3 * SLAB], BF16, tag="pw",
                                     name=f"pw{dt_}_{g}_{b}")
                        _powers(dA, dtbf_t[dt_], g, pw)
                    else:
                        for s in range(NSEG):
                            n = g * NSEG + s
                            nc.scalar.activation(seg(dA, s), dtbf_t[dt_][:], AF.Exp,
                                                 scale=aneg[dt_][:, n:n + 1])
                    dAv = dA[:].rearrange("p (s t) -> p s t", s=NSEG)
                    nc.vector.memset(dAv[:, :, 0:1], 0.0)
                    dBu = scn.tile([128, GW], BF16, tag="dBu", bufs=1,
                                   name=f"dBu{dt_}_{g}_{b}")
                    rep = dtu_t[dt_][:].unsqueeze(1).to_broadcast((128, NSEG, SEGW))
                    nc.vector.tensor_tensor(
                        dBu[:].rearrange("p (s t) -> p s t", s=NSEG),
                        rep, BBt[:].rearrange("p (s t) -> p s t", s=NSEG), OP.mult)
                    hloc = scn.tile([128, GW], BF16, tag="hloc")
                    nc.vector.tensor_tensor_scan(hloc[:], dA[:], dBu[:], 0.0,
                                                 OP.mult, OP.add)
                    hv = hloc[:].rearrange("p (s t) -> p s t", s=NSEG)
                    nc.vector.tensor_copy(
                        FDpk[:, dt_ * 32 + g * NSEG:dt_ * 32 + (g + 1) * NSEG],
                        hv[:, :, SEGW - 1:SEGW].rearrange("p s t -> p (s t)"))
                    # y partial: sum_n C*h via in-place tree on hC
                    hc = scn.tile([128, GW], BF16, tag="dBu", bufs=1,
                                  name=f"hc{dt_}_{g}_{b}")
                    nc.vector.tensor_tensor(hc[:], hloc[:], CCt[:], OP.mult)
                    nc.vector.tensor_tensor(hc[:, 0:4 * SEGW], hc[:, 0:4 * SEGW],
                                            hc[:, 4 * SEGW:], OP.add)
                    nc.vector.tensor_tensor(hc[:, 0:2 * SEGW], hc[:, 0:2 * SEGW],
                                            hc[:, 2 * SEGW:4 * SEGW], OP.add)
                    if g == 0:
                        nc.vector.tensor_tensor(y_[dt_][:], hc[:, 0:SEGW],
                                                hc[:, SEGW:2 * SEGW], OP.add)
                    else:
                        nc.vector.tensor_tensor(hc[:, 0:SEGW], hc[:, 0:SEGW],
                                                hc[:, SEGW:2 * SEGW], OP.add)
                        nc.vector.tensor_tensor(y_[dt_][:], y_[dt_][:],
                                                hc[:, 0:SEGW], OP.add)

            # ---------------- AG#4 + prefix chain ----------------
            ag4i = drp.tile([128, DT * 32], F32, tag=f"ag4i_{b}")
            ag4o = drp.tile([NCORES, 128, DT * 32], F32, tag=f"ag4o_{b}",
                            addr_space="Shared")
            nc.sync.dma_start(ag4i[:], FDpk[:])
            nc.gpsimd.collective_compute("AllGather", OP.bypass, replica_groups=RG,
                                         ins=[ag4i.opt()], outs=[ag4o.opt()])
            Spref = sm.tile([128, NCORES * DT * N16], F32, tag="Spref")
            scur = sm.tile([128, DT * N16], F32, tag="scur")
            nc.vector.memset(scur[:], 0.0)
            for r in range(NCORES):
                FDr = sm.tile([128, DT * 32], F32, tag="FDr", bufs=2,
                              name=f"FDr{r}_{b}")
                nc.sync.dma_start(FDr[:], ag4o[r])
                FDrv = FDr[:].rearrange("p (d c) -> p d c", d=DT)
                nc.vector.tensor_copy(Spref[:, r * DT * N16:(r + 1) * DT * N16],
                                      scur[:])
                sv = scur[:].rearrange("p (d c) -> p d c", d=DT)
                nc.vector.tensor_tensor(sv, FDrv[:, :, 16:32], sv, OP.mult)
                nc.vector.tensor_tensor(sv, sv, FDrv[:, :, 0:16], OP.add)
            h0all = Spref[:, bass.ts(pid, DT * N16)]
            h0v = h0all.rearrange("p (d c) -> p d c", d=DT)

            # ------------- correction: y += sum_n C * W^(n+1) * h0 -------------
            for g in range(NG):
                CCt = scn.tile([128, GW], BF16, tag="cc", bufs=1,
                               name=f"CCc{g}_{b}")
                for s in range(NSEG):
                    n = g * NSEG + s
                    nc.sync.dma_start(seg(CCt, s),
                                      bcd[N16 + n:N16 + n + 1, :].to_broadcast((128, SEGW)))
                for dt_ in range(DT):
                    h0 = h0v[:, dt_, :]
                    if debug and b == 0 and g == 0:
                        nc.sync.dma_start(dbg["d_h0"][dt_], h0)
                    P = scn.tile([128, GW], BF16, tag="dA", bufs=1,
                                 name=f"P{dt_}_{g}_{b}")
                    if apow:
                        pw = sm.tile([128, 3 * SLAB], BF16, tag="pw",
                                     name=f"pwc{dt_}_{g}_{b}")
                        _powers(P, Wt_t[dt_], g, pw)
                    else:
                        cdt = sm.tile([128, SLAB], F32, tag="cdt",
                                      name=f"cdt{dt_}_{g}_{b}")
                        nc.scalar.activation(cdt[:], Wt_t[dt_][:], AF.Ln)
                        for s in range(NSEG):
                            n = g * NSEG + s
                            nc.scalar.activation(seg(P, s), cdt[:], AF.Exp,
                                                 scale=aneg[dt_][:, n:n + 1])
                    for s in range(NSEG):
                        nc.vector.scalar_tensor_tensor(
                            seg(P, s), seg(P, s), h0[:, g * NSEG + s:g * NSEG + s + 1],
                            seg(CCt, s), OP.mult, OP.mult)
                    nc.vector.tensor_tensor(P[:, 0:4 * SEGW], P[:, 0:4 * SEGW],
                                            P[:, 4 * SEGW:], OP.add)
                    nc.vector.tensor_tensor(P[:, 0:2 * SEGW], P[:, 0:2 * SEGW],
                                            P[:, 2 * SEGW:4 * SEGW], OP.add)
                    nc.vector.tensor_tensor(P[:, 0:SEGW], P[:, 0:SEGW],
                                            P[:, SEGW:2 * SEGW], OP.add)
                    nc.vector.tensor_tensor(y_[dt_][:], y_[dt_][:], P[:, 0:SEGW],
                                            OP.add)

            # ---------------- gate + out_proj ----------------
            yf_ = []
            for dt_ in range(DT):
                if debug and b == 0:
                    nc.sync.dma_start(dbg["d_y"][dt_], y_[dt_][:])
                yd = sm.tile([128, SLAB], F32, tag="yd")
                nc.vector.tensor_scalar(yd[:], u_[dt_][:], dsk[dt_][:], 0.0,
                                        OP.mult, OP.add)
                nc.vector.tensor_tensor(yd[:], y_[dt_][:], yd[:], OP.add)
                yf = u_[dt_]
                nc.vector.tensor_tensor(yf[:], yd[:], g_[dt_][:], OP.mult)
                yf_.append(yf)
            for mt in range(CT):
                psum = psP.tile([128, SLAB], F32, tag="convps", bufs=2)
                for kt in range(DT):
                    nc.tensor.matmul(psum[:], ow[kt][mt], yf_[kt][:],
                                     start=(kt == 0), stop=(kt == DT - 1))
                o_sb = sm.tile([128, SLAB], F32, tag="osb")
                nc.scalar.copy(o_sb[:], psum[:])
                nc.sync.dma_start(out_t[b, mt], o_sb[:])

    nc.finalize()
    _BUILT[key] = nc
    return nc


# ======================================================================
# entry point
# ======================================================================

def _install_trace_hook():
    """The container's antenv lacks axon_hooks; synthesize it and install the
    NTFF profiling hook so trace=True yields exec_time_ns."""
    import types, sys as _sys
    try:
        import antenv.axon_hooks  # noqa
        return
    except ImportError:
        pass
    mod = types.ModuleType("antenv.axon_hooks")
    mod._hook = None
    def set_axon_ntff_profile_hook(h):
        mod._hook = h
    def get_axon_ntff_profile_hook():
        return mod._hook
    mod.set_axon_ntff_profile_hook = set_axon_ntff_profile_hook
    mod.get_axon_ntff_profile_hook = get_axon_ntff_profile_hook
    _sys.modules["antenv.axon_hooks"] = mod
    try:
        import antenv
        antenv.axon_hooks = mod
    except ImportError:
        pass
    try:
        from trn_agent_boot.trn_boot import _ntff_profile_via_ctypes
        hk = _ntff_profile_via_ctypes("/opt/axon/libaxon_pjrt.so")
        if hk is not None:
            mod._hook = hk
    except Exception as e:
        print(f"trace hook install failed: {e}")


def _a_is_structured(inputs):
    A = -np.exp(_f32(inputs["A_log"]))
    ref = -np.arange(1, N16 + 1, dtype=np.float32)
    return bool(np.abs(A - ref[None, :]).max() < 1e-4)


def kernel(**inputs):
    from concourse.bass_utils import run_bass_kernel_spmd
    if os.environ.get("K_TRACE"):
        _install_trace_hook()
    nc = build_nc(debug=bool(os.environ.get("K_DEBUG")),
                  apow=_a_is_structured(inputs))
    in_maps = prep_inputs(inputs)
    res = run_bass_kernel_spmd(nc, in_maps, core_ids=list(range(NCORES)),
                               trace=bool(os.environ.get("K_TRACE")))
    out = np.zeros((B, C, L), np.float32)
    for j in range(NCORES):
        out[:, :, j * SLAB:(j + 1) * SLAB] = \
            res.results[j]["out"].reshape(B, C, SLAB)
    if os.environ.get("K_DEBUG"):
        kernel.dbg = res.results
    kernel.exec_time_ns = res.exec_time_ns
    return out.reshape(B, C, Dd, H, W)

